# revision 23
# baseline (speedup 1.0000x reference)
"""Trainium2 Bass kernel for nn_DimBlock_1 (light-field 4D conv -> 2D conv).

Math: out[b, oc, h, w] = bias[oc] +
      sum_{ic<25, kh<9, kw<9} pic[b, ic, h+kh, w+kw] * W[oc, ic, kh, kw]
with pic [8, 25, 256, 256] (25 = 5x5 angular dims folded to channels),
W [100, 25, 9, 9], output [8, 100, 1, 1, 248, 248].

Strategy (pure data parallel, 1 image per NeuronCore):
- Flatten the image spatially: free dim = h*256+w. A matmul's moving AP
  [125, (256, 2), (1, 248)] covers one PSUM tile of 2 output rows x 248
  columns (N=496 <= 512 fp32 bank) with zero width overcompute.
- Pack the contraction into partitions: 25 channels x 5 shifted copies.
  TWO replicas of the image live in DRAM: picH shifts groups by {0..4}
  pixels (horizontal), picV by {0..4} ROWS (vertical). One K=125 matmul
  then covers 5 horizontally- OR vertically-adjacent kernel taps.
- The 9x9 tap grid is tiled with 17 pieces (the provable minimum for
  <=5-tap pieces): rows 0-4 as 9 vertical 1x5 pieces (full 125-K), rows
  5-8 as 8 horizontal pieces (4 full + 4 with 100-K). 17 accumulating
  matmuls per PSUM tile vs 18 for the pure-horizontal split: the PE
  ingests one moving column per cycle, so this is a direct 5.6% cut in
  the dominant cost (17 x 496 x 124 tiles ~= 436 us PE floor).
- bf16 operands (rel err ~3e-3, gate 2e-2) and bf16 output staging
  (host upcasts): halves both input and output HBM traffic.
- A warmup ladder of dummy matmuls on zeroed SBUF (8 at N=496, then 14
  at N=128) keeps the PE busy from t~0 so the HAM clock gate reaches
  8/8 (2.4 GHz) before the first real matmul and stays there until the
  first strip's data lands; without it the first ~11 real matmuls run
  at 1.2 GHz (and a PE-idle gap >3.4 us would re-throttle it).
- First-iteration latency: strip-0's tiny picV slice (133 KB) and the
  j=0 weight column (26 KB) are DMAed first on separate queues, so the
  first real matmul waits only on them, not the full weight+strip
  load. First-DMA completion is latency-bound at ~4.3 us after the
  ~7.2 us engine-barrier preamble, so real work starts ~11.9 us in.
- Matmuls run j-outer over groups of up to 8 PSUM banks; evictions are
  a DVE tensor_scalar add of the per-partition bias so the Activation
  engine's instruction stream stays free for input DMA issue.
- Output staging is packed 248-wide bf16; strips drain as contiguous
  descriptors per partition via the software DGE, the last three via
  the then-idle HWDGE queues to shorten the final tail.
"""

import sys

sys.path.insert(0, "/opt/trn_rl_repo")

import ml_dtypes
import numpy as np

from concourse import bacc
import concourse.tile as tile
import concourse.mybir as mybir
from concourse.bass_utils import run_bass_kernel_spmd

B, C, H, W = 8, 25, 256, 256
OC, KH, KW = 100, 9, 9
OH, OW = H - KH + 1, W - KW + 1  # 248, 248
NCORES = 8
NPIX = H * W

STRIP = 16              # output rows per main strip
NMM = 15                # bf16 matmuls per tile: 9 vertical + 6 horizontal
HJS = [(5, 5), (6, 5), (7, 0), (7, 5), (8, 0), (8, 5)]  # bf16 H pieces (kh, kwb)
W8STEP = 112            # fp8 stationary: halves at 0 and 112 (16B-aligned)
F8SCALE = 7.0           # balanced fp8 scaling: w*7, x/7 (keeps both out of
                        # the e4m3 denormal range in the common case)
KP = 125                # contraction partitions: 25 ch x 5 shift groups
PAD = 16                # zero pad per image so shifted copies stay in bounds
NPIXP = NPIX + PAD
GRP = 8                 # psum tiles per j-outer matmul group
NWARM = 8               # big dummy matmuls to pull HAM to 8/8
NWARM_SMALL = 14        # small dummies bridging until real data lands

F32 = mybir.dt.float32
BF16 = mybir.dt.bfloat16
F8 = mybir.dt.float8e4
NP_BF16 = ml_dtypes.bfloat16
NP_F8 = ml_dtypes.float8_e4m3

_compiled = None


def _need_v(rows):
    # vertical pieces: base (2t-h0)*W + kw, kw<=8, reads to base+W+OW-1
    return rows * W + 8


def _need_h(rows):
    # horizontal pieces: tile starts at image row h0+5; base
    # (2t+kh-h0-5)*W + kwb, max (rows+1)*W+5, reads to +W+OW-1
    return (rows + 2) * W + 253


def _build():
    nc = bacc.Bacc("TRN2", target_bir_lowering=False, debug=False,
                   num_devices=NCORES)
    pich = nc.dram_tensor("pich", [128, NPIXP], BF16, kind="ExternalInput").ap()
    picv = nc.dram_tensor("picv", [128, NPIXP], BF16, kind="ExternalInput").ap()
    pich8 = nc.dram_tensor("pich8", [128, NPIXP], F8,
                           kind="ExternalInput").ap()
    wp = nc.dram_tensor("wp", [128, NMM, OC], BF16, kind="ExternalInput").ap()
    wp8 = nc.dram_tensor("wp8", [128, 224], F8, kind="ExternalInput").ap()
    bias = nc.dram_tensor("bias", [OC, 1], F32, kind="ExternalInput").ap()
    out = nc.dram_tensor("out", [OC, OH, OW], BF16, kind="ExternalOutput").ap()

    with tile.TileContext(nc) as tc:
        with (
            tc.tile_pool(name="wpool", bufs=1) as wpool,
            tc.tile_pool(name="vpool", bufs=4) as vpool,
            tc.tile_pool(name="hpool", bufs=4) as hpool,
            tc.tile_pool(name="h8pool", bufs=4) as h8pool,
            tc.tile_pool(name="outpool", bufs=3) as outpool,
            tc.tile_pool(name="pspool", bufs=8, space="PSUM") as pspool,
        ):
            # small first/last strips shorten pipeline fill and drain
            strip_sizes = [2, 4, 8] + [STRIP] * 13 + [8, 8, 6, 2, 2]
            assert sum(strip_sizes) == OH

            # ---- PE warmup: dummy matmuls on zeroed SBUF from t~0 so the
            # HAM clock gate is at 8/8 before the first real matmul. The
            # dummy PSUM bank is recycled by the pool; real matmuls
            # overwrite it with start=True.
            wu = wpool.tile([128, 512], BF16)
            nc.vector.memset(wu[:], 0.0)
            wupt = pspool.tile([OC, 2 * OW], F32, tag="pt", name="pt")
            # ladder: big dummies warm HAM, then small N=128 ones keep
            # the PE busy in fine steps until the first strip's data
            # lands, whenever that is (early-DMA latency varies)
            for _ in range(NWARM):
                nc.tensor.matmul(wupt[:, 0:496], wu[0:KP, 0:100],
                                 wu[0:KP, 0:496], start=True, stop=True)
            for _ in range(NWARM_SMALL):
                nc.tensor.matmul(wupt[:, 0:128], wu[0:KP, 0:100],
                                 wu[0:KP, 0:128], start=True, stop=True)

            # ---- initial loads, finest-granularity first: the j=0 matmul
            # (vertical piece, kw=0) needs only strip-0's picv slice and
            # the j=0 weight column.
            rows0 = strip_sizes[0]
            nv0 = _need_v(rows0)
            nh0 = _need_h(rows0)
            vt0 = vpool.tile([128, _need_v(STRIP)], BF16, tag="vt", name="vt")
            ht0 = hpool.tile([128, _need_h(STRIP)], BF16, tag="ht", name="ht")
            h80 = h8pool.tile([128, _need_h(STRIP)], F8, tag="h8", name="h8")
            nc.sync.dma_start(vt0[0:128, 0:nv0], picv[0:128, 0:nv0])
            wt = wpool.tile([128, NMM, OC], BF16)
            nc.scalar.dma_start(wt[:, 0:1], wp[:, 0:1])
            nc.scalar.dma_start(wt[:, 1:9], wp[:, 1:9])
            nc.sync.dma_start(ht0[0:128, 0:nh0],
                              pich[0:128, 5 * W:5 * W + nh0])
            nc.scalar.dma_start(wt[:, 9:NMM], wp[:, 9:NMM])
            wt8 = wpool.tile([128, 224], F8)
            bt = wpool.tile([OC, 1], F32)
            nc.scalar.dma_start(bt[:], bias[:])
            nc.sync.dma_start(h80[0:128, 0:nh0],
                              pich8[0:128, 5 * W:5 * W + nh0])
            nc.scalar.dma_start(wt8[:], wp8[:])

            h0 = 0
            for si, rows in enumerate(strip_sizes):
                nv, nh = _need_v(rows), _need_h(rows)
                if si == 0:
                    vt, ht, h8 = vt0, ht0, h80
                else:
                    vt = vpool.tile([128, _need_v(STRIP)], BF16,
                                    tag="vt", name="vt")
                    ht = hpool.tile([128, _need_h(STRIP)], BF16,
                                    tag="ht", name="ht")
                    h8 = h8pool.tile([128, _need_h(STRIP)], F8,
                                     tag="h8", name="h8")
                    # split each strip across both HWDGE queues; 64-partition
                    # halves so each gets the full 16-engine fan-out
                    vb = h0 * W
                    hb = (h0 + 5) * W
                    nc.sync.dma_start(vt[0:64, 0:nv], picv[0:64, vb:vb + nv])
                    nc.scalar.dma_start(vt[64:128, 0:nv],
                                        picv[64:128, vb:vb + nv])
                    nc.sync.dma_start(ht[0:64, 0:nh], pich[0:64, hb:hb + nh])
                    nc.scalar.dma_start(ht[64:128, 0:nh],
                                        pich[64:128, hb:hb + nh])
                    nc.sync.dma_start(h8[0:64, 0:nh],
                                      pich8[0:64, hb:hb + nh])
                    nc.scalar.dma_start(h8[64:128, 0:nh],
                                        pich8[64:128, hb:hb + nh])
                ot = outpool.tile([OC, STRIP * OW], BF16, tag="ot")
                ntiles = rows // 2   # one 496-px psum tile per 2 output rows
                for g0 in range(0, ntiles, GRP):
                    gts = list(range(g0, min(g0 + GRP, ntiles)))
                    pts = [pspool.tile([OC, 2 * OW], F32, tag="pt", name="pt")
                           for _ in gts]
                    for j in range(NMM + 1):
                        for pt, t in zip(pts, gts):
                            if j == NMM:
                                # fp8 DoubleRow: pieces (kh5, kw0-4) and
                                # (kh6, kw0-4) fused; Ko halves 256 apart.
                                # h8 starts at image row h0+5, so the local
                                # base for tap kh=5 of local row 2t is 2t*W
                                o = 2 * t * W
                                rhs = h8[0:KP, o:o + 3 * W + OW].copy()
                                ps = rhs.ap[0][0]
                                rhs.ap = mybir.VecI64Pair(
                                    [[ps, KP], [W, 2], [W, 2], [1, OW]])
                                lhs = wt8[0:KP, 0:W8STEP + OC].copy()
                                qs = lhs.ap[0][0]
                                lhs.ap = mybir.VecI64Pair(
                                    [[qs, KP], [W8STEP, 2], [1, OC]])
                                nc.tensor.matmul(
                                    pt[:], lhs, rhs, start=False, stop=True,
                                    perf_mode=mybir.MatmulPerfMode.DoubleRow)
                                continue
                            # 3D moving AP: 2 rows of 248 useful columns
                            if j < 9:
                                src, o = vt, 2 * t * W + j
                            else:
                                kh, kwb = HJS[j - 9]
                                src, o = ht, (2 * t + kh - 5) * W + kwb
                            rhs = src[0:KP, o:o + W + OW].copy()
                            ps = rhs.ap[0][0]
                            rhs.ap = mybir.VecI64Pair(
                                [[ps, KP], [W, 2], [1, OW]])
                            nc.tensor.matmul(pt[:], wt[0:KP, j, :], rhs,
                                             start=(j == 0), stop=False)
                    for pt, t in zip(pts, gts):
                        # evict on DVE: keeps the Activation engine free to
                        # issue input DMAs
                        nc.vector.tensor_scalar_add(
                            ot[:, t * 2 * OW:(t + 1) * 2 * OW], pt[:], bt[:])
                # packed 248-wide rows: one contiguous descriptor per
                # partition per strip. last strips drain via the HWDGE
                # queues (idle once input prefetch is done), split across
                # both by partition halves to shorten the final tail
                if si >= len(strip_sizes) - 3:
                    nc.sync.dma_start(out[0:50, h0:h0 + rows, :],
                                      ot[0:50, :rows * OW])
                    nc.scalar.dma_start(out[50:OC, h0:h0 + rows, :],
                                        ot[50:OC, :rows * OW])
                else:
                    nc.gpsimd.dma_start(out[:, h0:h0 + rows, :],
                                        ot[:, :rows * OW])
                h0 += rows

    nc.compile()
    return nc


def _pack_weights(weight: np.ndarray):
    w2 = np.ascontiguousarray(weight.reshape(OC, C, KH, KW))
    wpk = np.zeros((128, NMM, OC), dtype=np.float32)
    for j in range(9):           # vertical pieces: kw=j, kh=g
        for g in range(5):
            wpk[25 * g:25 * g + 25, j, :] = w2[:, :, g, j].T
    for j in range(9, NMM):      # bf16 horizontal pieces
        kh, kwb = HJS[j - 9]
        for g in range(5):
            kw = kwb + g
            if kw < KW:
                wpk[25 * g:25 * g + 25, j, :] = w2[:, :, kh, kw].T
    # fp8 DoubleRow weights: (kh5, kw g) half at 0, (kh6, kw g) at W8STEP
    w8 = np.zeros((128, 224), dtype=np.float32)
    for g in range(5):
        w8[25 * g:25 * g + 25, 0:OC] = F8SCALE * w2[:, :, 5, g].T
        w8[25 * g:25 * g + 25, W8STEP:W8STEP + OC] = \
            F8SCALE * w2[:, :, 6, g].T
    return wpk.astype(NP_BF16), w8.astype(NP_F8)


def _replicate_pics(pic: np.ndarray):
    """[B, C, NPIX] -> two [B, 128, NPIXP] replicas: 5 copies of the 25
    channels shifted by {0..4} pixels (H) and by {0..4} rows (V)."""
    pich = np.zeros((B, 128, NPIXP), dtype=NP_BF16)
    picv = np.zeros((B, 128, NPIXP), dtype=NP_BF16)
    pich8 = np.zeros((B, 128, NPIXP), dtype=NP_F8)
    pic8 = pic / F8SCALE
    for g in range(5):
        pich[:, 25 * g:25 * g + 25, 0:NPIX - g] = pic[:, :, g:]
        picv[:, 25 * g:25 * g + 25, 0:NPIX - g * W] = pic[:, :, g * W:]
        pich8[:, 25 * g:25 * g + 25, 0:NPIX - g] = pic8[:, :, g:]
    return pich, picv, pich8


def _run(pic_in, weight, bias, trace=False):
    global _compiled
    if _compiled is None:
        _compiled = _build()
    nc = _compiled
    wpk, w8 = _pack_weights(np.asarray(weight, dtype=np.float32))
    bvec = np.ascontiguousarray(
        np.asarray(bias, dtype=np.float32).reshape(OC, 1))
    pic = np.asarray(pic_in, dtype=np.float32).reshape(B, C, NPIX)
    pich, picv, pich8 = _replicate_pics(pic)
    in_maps = [
        {"pich": pich[i], "picv": picv[i], "pich8": pich8[i],
         "wp": wpk, "wp8": w8, "bias": bvec}
        for i in range(NCORES)
    ]
    res = run_bass_kernel_spmd(nc, in_maps, core_ids=list(range(NCORES)),
                               trace=trace)
    full = np.stack([res.results[i]["out"] for i in range(NCORES)], axis=0)
    return full.astype(np.float32).reshape(B, OC, 1, 1, OH, OW), res


def kernel(pic_in, weight, bias):
    out, _ = _run(pic_in, weight, bias, trace=False)
    return out


def kernel_traced(pic_in, weight, bias):
    return _run(pic_in, weight, bias, trace=True)


# revision 24
# speedup vs baseline: 1.1289x; 1.1289x over previous
"""Trainium2 Bass kernel for nn_DimBlock_1 (light-field 4D conv -> 2D conv).

Math: out[b, oc, h, w] = bias[oc] +
      sum_{ic<25, kh<9, kw<9} pic[b, ic, h+kh, w+kw] * W[oc, ic, kh, kw]
with pic [8, 25, 256, 256] (25 = 5x5 angular dims folded to channels),
W [100, 25, 9, 9], output [8, 100, 1, 1, 248, 248].

Strategy (pure data parallel, 1 image per NeuronCore):
- Flatten the image spatially: free dim = h*256+w. A matmul's moving AP
  [125, (256, 2), (1, 248)] covers one PSUM tile of 2 output rows x 248
  columns (N=496 <= 512 fp32 bank) with zero width overcompute.
- Pack the contraction into partitions: 25 channels x 5 shifted copies.
  TWO replicas of the image live in DRAM: picH shifts groups by {0..4}
  pixels (horizontal), picV by {0..4} ROWS (vertical). One K=125 matmul
  then covers 5 horizontally- OR vertically-adjacent kernel taps.
- The 9x9 tap grid is tiled with 17 pieces (the provable minimum for
  <=5-tap pieces): rows 0-4 as 9 vertical 1x5 pieces (full 125-K), rows
  5-8 as 8 horizontal pieces (4 full + 4 with 100-K). 17 accumulating
  matmuls per PSUM tile vs 18 for the pure-horizontal split: the PE
  ingests one moving column per cycle, so this is a direct 5.6% cut in
  the dominant cost (17 x 496 x 124 tiles ~= 436 us PE floor).
- bf16 operands (rel err ~3e-3, gate 2e-2) and bf16 output staging
  (host upcasts): halves both input and output HBM traffic.
- A warmup ladder of dummy matmuls on zeroed SBUF (8 at N=496, then 14
  at N=128) keeps the PE busy from t~0 so the HAM clock gate reaches
  8/8 (2.4 GHz) before the first real matmul and stays there until the
  first strip's data lands; without it the first ~11 real matmuls run
  at 1.2 GHz (and a PE-idle gap >3.4 us would re-throttle it).
- First-iteration latency: strip-0's tiny picV slice (133 KB) and the
  j=0 weight column (26 KB) are DMAed first on separate queues, so the
  first real matmul waits only on them, not the full weight+strip
  load. First-DMA completion is latency-bound at ~4.3 us after the
  ~7.2 us engine-barrier preamble, so real work starts ~11.9 us in.
- Matmuls run j-outer over groups of up to 8 PSUM banks; evictions are
  a DVE tensor_scalar add of the per-partition bias so the Activation
  engine's instruction stream stays free for input DMA issue.
- Output staging is packed 248-wide bf16; strips drain as contiguous
  descriptors per partition via the software DGE, the last three via
  the then-idle HWDGE queues to shorten the final tail.
"""

import sys

sys.path.insert(0, "/opt/trn_rl_repo")

import ml_dtypes
import numpy as np

from concourse import bacc
import concourse.tile as tile
import concourse.mybir as mybir
from concourse.bass_utils import run_bass_kernel_spmd

B, C, H, W = 8, 25, 256, 256
OC, KH, KW = 100, 9, 9
OH, OW = H - KH + 1, W - KW + 1  # 248, 248
NCORES = 8
NPIX = H * W

STRIP = 16              # output rows per main strip
NMM = 17                # matmuls per psum tile: 9 vertical + 8 horizontal
KP = 125                # contraction partitions: 25 ch x 5 shift groups
PAD = 16                # zero pad per image so shifted copies stay in bounds
NPIXP = NPIX + PAD
GRP = 8                 # psum tiles per j-outer matmul group
NWARM = 8               # big dummy matmuls to pull HAM to 8/8
NWARM_SMALL = 14        # small dummies bridging until real data lands

F32 = mybir.dt.float32
BF16 = mybir.dt.bfloat16
NP_BF16 = ml_dtypes.bfloat16

_compiled = None


def _need_v(rows):
    # vertical pieces: base (2t-h0)*W + kw, kw<=8, reads to base+W+OW-1
    return rows * W + 8


def _need_h(rows):
    # horizontal pieces: tile starts at image row h0+5; base
    # (2t+kh-h0-5)*W + kwb, max (rows+1)*W+5, reads to +W+OW-1
    return (rows + 2) * W + 253


def _build():
    nc = bacc.Bacc("TRN2", target_bir_lowering=False, debug=False,
                   num_devices=NCORES)
    pich = nc.dram_tensor("pich", [128, NPIXP], BF16, kind="ExternalInput").ap()
    picv = nc.dram_tensor("picv", [128, NPIXP], BF16, kind="ExternalInput").ap()
    wp = nc.dram_tensor("wp", [128, NMM, OC], BF16, kind="ExternalInput").ap()
    bias = nc.dram_tensor("bias", [OC, 1], F32, kind="ExternalInput").ap()
    out = nc.dram_tensor("out", [OC, OH, OW], BF16, kind="ExternalOutput").ap()

    with tile.TileContext(nc) as tc:
        with (
            tc.tile_pool(name="wpool", bufs=1) as wpool,
            tc.tile_pool(name="vpool", bufs=4) as vpool,
            tc.tile_pool(name="hpool", bufs=4) as hpool,
            tc.tile_pool(name="outpool", bufs=3) as outpool,
            tc.tile_pool(name="pspool", bufs=8, space="PSUM") as pspool,
        ):
            # small first/last strips shorten pipeline fill and drain
            strip_sizes = [2, 4, 8] + [STRIP] * 13 + [8, 8, 6, 2, 2]
            assert sum(strip_sizes) == OH

            # ---- PE warmup: dummy matmuls on zeroed SBUF from t~0 so the
            # HAM clock gate is at 8/8 before the first real matmul. The
            # dummy PSUM bank is recycled by the pool; real matmuls
            # overwrite it with start=True.
            wu = wpool.tile([128, 512], BF16)
            nc.vector.memset(wu[:], 0.0)
            wupt = pspool.tile([OC, 2 * OW], F32, tag="pt", name="pt")
            # ladder: big dummies warm HAM, then small N=128 ones keep
            # the PE busy in fine steps until the first strip's data
            # lands, whenever that is (early-DMA latency varies)
            for _ in range(NWARM):
                nc.tensor.matmul(wupt[:, 0:496], wu[0:KP, 0:100],
                                 wu[0:KP, 0:496], start=True, stop=True)
            for _ in range(NWARM_SMALL):
                nc.tensor.matmul(wupt[:, 0:128], wu[0:KP, 0:100],
                                 wu[0:KP, 0:128], start=True, stop=True)

            # ---- initial loads, finest-granularity first: the j=0 matmul
            # (vertical piece, kw=0) needs only strip-0's picv slice and
            # the j=0 weight column.
            rows0 = strip_sizes[0]
            nv0 = _need_v(rows0)
            nh0 = _need_h(rows0)
            vt0 = vpool.tile([128, _need_v(STRIP)], BF16, tag="vt", name="vt")
            ht0 = hpool.tile([128, _need_h(STRIP)], BF16, tag="ht", name="ht")
            nc.sync.dma_start(vt0[0:128, 0:nv0], picv[0:128, 0:nv0])
            wt = wpool.tile([128, NMM, OC], BF16)
            nc.scalar.dma_start(wt[:, 0:1], wp[:, 0:1])
            nc.scalar.dma_start(wt[:, 1:9], wp[:, 1:9])
            nc.sync.dma_start(ht0[0:128, 0:nh0],
                              pich[0:128, 5 * W:5 * W + nh0])
            nc.scalar.dma_start(wt[:, 9:NMM], wp[:, 9:NMM])
            bt = wpool.tile([OC, 1], F32)
            nc.scalar.dma_start(bt[:], bias[:])

            h0 = 0
            for si, rows in enumerate(strip_sizes):
                nv, nh = _need_v(rows), _need_h(rows)
                if si == 0:
                    vt, ht = vt0, ht0
                else:
                    vt = vpool.tile([128, _need_v(STRIP)], BF16,
                                    tag="vt", name="vt")
                    ht = hpool.tile([128, _need_h(STRIP)], BF16,
                                    tag="ht", name="ht")
                    # split each strip across both HWDGE queues; 64-partition
                    # halves so each gets the full 16-engine fan-out
                    vb = h0 * W
                    hb = (h0 + 5) * W
                    nc.sync.dma_start(vt[0:64, 0:nv], picv[0:64, vb:vb + nv])
                    nc.scalar.dma_start(vt[64:128, 0:nv],
                                        picv[64:128, vb:vb + nv])
                    nc.sync.dma_start(ht[0:64, 0:nh], pich[0:64, hb:hb + nh])
                    nc.scalar.dma_start(ht[64:128, 0:nh],
                                        pich[64:128, hb:hb + nh])
                ot = outpool.tile([OC, STRIP * OW], BF16, tag="ot")
                ntiles = rows // 2   # one 496-px psum tile per 2 output rows
                for g0 in range(0, ntiles, GRP):
                    gts = list(range(g0, min(g0 + GRP, ntiles)))
                    pts = [pspool.tile([OC, 2 * OW], F32, tag="pt", name="pt")
                           for _ in gts]
                    for j in range(NMM):
                        for pt, t in zip(pts, gts):
                            # 3D moving AP: 2 rows of 248 useful columns
                            if j < 9:
                                src, o = vt, 2 * t * W + j
                            else:
                                kh = 5 + (j - 9) // 2
                                kwb = ((j - 9) % 2) * 5
                                src, o = ht, (2 * t + kh - 5) * W + kwb
                            rhs = src[0:KP, o:o + W + OW].copy()
                            ps = rhs.ap[0][0]
                            rhs.ap = mybir.VecI64Pair(
                                [[ps, KP], [W, 2], [1, OW]])
                            nc.tensor.matmul(pt[:], wt[0:KP, j, :], rhs,
                                             start=(j == 0),
                                             stop=(j == NMM - 1))
                    for pt, t in zip(pts, gts):
                        # evict on DVE: keeps the Activation engine free to
                        # issue input DMAs
                        nc.vector.tensor_scalar_add(
                            ot[:, t * 2 * OW:(t + 1) * 2 * OW], pt[:], bt[:])
                # packed 248-wide rows: one contiguous descriptor per
                # partition per strip. last strips drain via the HWDGE
                # queues (idle once input prefetch is done), split across
                # both by partition halves to shorten the final tail
                if si >= len(strip_sizes) - 3:
                    nc.sync.dma_start(out[0:50, h0:h0 + rows, :],
                                      ot[0:50, :rows * OW])
                    nc.scalar.dma_start(out[50:OC, h0:h0 + rows, :],
                                        ot[50:OC, :rows * OW])
                else:
                    nc.gpsimd.dma_start(out[:, h0:h0 + rows, :],
                                        ot[:, :rows * OW])
                h0 += rows

    nc.compile()
    return nc


def _pack_weights(weight: np.ndarray) -> np.ndarray:
    w2 = np.ascontiguousarray(weight.reshape(OC, C, KH, KW))
    wpk = np.zeros((128, NMM, OC), dtype=np.float32)
    for j in range(9):           # vertical pieces: kw=j, kh=g
        for g in range(5):
            wpk[25 * g:25 * g + 25, j, :] = w2[:, :, g, j].T
    for j in range(9, NMM):      # horizontal pieces: kh 5..8, kw halves
        kh = 5 + (j - 9) // 2
        kwb = ((j - 9) % 2) * 5
        for g in range(5):
            kw = kwb + g
            if kw < KW:
                wpk[25 * g:25 * g + 25, j, :] = w2[:, :, kh, kw].T
    return wpk.astype(NP_BF16)


def _replicate_pics(pic: np.ndarray):
    """[B, C, NPIX] -> two [B, 128, NPIXP] replicas: 5 copies of the 25
    channels shifted by {0..4} pixels (H) and by {0..4} rows (V)."""
    pich = np.zeros((B, 128, NPIXP), dtype=NP_BF16)
    picv = np.zeros((B, 128, NPIXP), dtype=NP_BF16)
    for g in range(5):
        pich[:, 25 * g:25 * g + 25, 0:NPIX - g] = pic[:, :, g:]
        picv[:, 25 * g:25 * g + 25, 0:NPIX - g * W] = pic[:, :, g * W:]
    return pich, picv


def _run(pic_in, weight, bias, trace=False):
    global _compiled
    if _compiled is None:
        _compiled = _build()
    nc = _compiled
    wpk = _pack_weights(np.asarray(weight, dtype=np.float32))
    bvec = np.ascontiguousarray(
        np.asarray(bias, dtype=np.float32).reshape(OC, 1))
    pic = np.asarray(pic_in, dtype=np.float32).reshape(B, C, NPIX)
    pich, picv = _replicate_pics(pic.astype(NP_BF16))
    in_maps = [
        {"pich": pich[i], "picv": picv[i], "wp": wpk, "bias": bvec}
        for i in range(NCORES)
    ]
    res = run_bass_kernel_spmd(nc, in_maps, core_ids=list(range(NCORES)),
                               trace=trace)
    full = np.stack([res.results[i]["out"] for i in range(NCORES)], axis=0)
    return full.astype(np.float32).reshape(B, OC, 1, 1, OH, OW), res


def kernel(pic_in, weight, bias):
    out, _ = _run(pic_in, weight, bias, trace=False)
    return out


def kernel_traced(pic_in, weight, bias):
    return _run(pic_in, weight, bias, trace=True)
